# revision 1
# baseline (speedup 1.0000x reference)
"""EulerGCN on 8 trn2 NeuronCores — two SPMD launches.

K1: core t owns snapshot t. table1 = dinv*(x@W1) on device; 2 GCN props
    via ELL gathers + DVE tree reduce + unique-idx scatter-add into
    natural-order accumulators; relu/W2/tanh finish. Output tanhE [NPAD,H].
K2: node-sharded GRU + linear head.

Host does integer layout (edge grouping, degree sort, idx grids) and the
standard GCN normalization coefficients dinv = rsqrt(deg).
"""

import numpy as np
import concourse.bass as bass
import concourse.bacc as bacc
import concourse.mybir as mybir
import concourse.tile as tile
from concourse.bass_utils import run_bass_kernel_spmd
from concourse.masks import make_identity

P = 128
NCORES = 8
N = 100000
NPAD = 100352           # 784 blocks of 128; 4 quarters of 25088
QN = NPAD // 4          # 25088
NBLK = NPAD // P        # 784
T = 8
XD = 128
H = 64
Z = 32
ACCR = QN + P           # accumulator rows per quarter (+dummy block)
F32 = mybir.dt.float32
I16 = mybir.dt.int16
NSH = NPAD // NCORES    # 12544 nodes per core in K2
JC = NSH // P           # 98 columns


def wrap16(a):
    n = a.shape[0]
    return np.ascontiguousarray(np.tile(a.reshape(n // 16, 16).T, (8, 1)))


def build_structure(ei, ew, dinv):
    """Per-snapshot host structure: 16 (dst-quarter r, src-quarter q)
    sections; nodes degree-sorted per section; returns per-section
    per-block grids as streams."""
    src = ei[0].astype(np.int64)
    dst = ei[1].astype(np.int64)
    w = (ew * dinv[dst]).astype(np.float32)  # dinv[src] folded into table
    out = []
    for r in range(4):
        for q in range(4):
            m = (dst // QN == r) & (src // QN == q)
            s_src = src[m] - q * QN
            s_dst = dst[m] - r * QN
            s_w = w[m]
            cnt = np.bincount(s_dst, minlength=QN)
            order = np.argsort(-cnt, kind="stable")
            nactive = int((cnt > 0).sum())
            nact_pad = max(P, ((nactive + P - 1) // P) * P)
            nodes = order[:nact_pad]
            nblocks = nact_pad // P
            counts = cnt[nodes].astype(np.int64)
            Ls = counts.reshape(nblocks, P).max(axis=1).astype(np.int32)
            rank_of = np.full(QN, -1, np.int64)
            rank_of[nodes] = np.arange(nact_pad)
            erank = rank_of[s_dst]
            eorder = np.argsort(erank, kind="stable")
            er_sorted = erank[eorder]
            slot = np.arange(er_sorted.size) - np.searchsorted(er_sorted, er_sorted)
            out.append({
                "nodes": nodes, "Ls": Ls, "nblocks": nblocks,
                "e_src": s_src[eorder], "e_w": s_w[eorder],
                "e_rank": er_sorted, "e_slot": slot,
            })
    return out


def kernel(**inputs):
    x = np.asarray(inputs["x"], np.float32)
    eis = np.asarray(inputs["eis"])
    ews = np.asarray(inputs["ews"], np.float32)
    W1 = np.asarray(inputs["W1"], np.float32)
    b1 = np.asarray(inputs["b1"], np.float32)
    W2 = np.asarray(inputs["W2"], np.float32)
    b2 = np.asarray(inputs["b2"], np.float32)
    Wih = np.asarray(inputs["Wih"], np.float32)
    Whh = np.asarray(inputs["Whh"], np.float32)
    bih = np.asarray(inputs["bih"], np.float32)
    bhh = np.asarray(inputs["bhh"], np.float32)
    Wlin = np.asarray(inputs["Wlin"], np.float32)
    blin = np.asarray(inputs["blin"], np.float32)

    # ---------------- host layout ----------------
    dinvs = []
    for t in range(T):
        deg = np.bincount(eis[t, 1].astype(np.int64), weights=ews[t],
                          minlength=N).astype(np.float32) + 1.0
        dinv = (1.0 / np.sqrt(deg)).astype(np.float32)
        dinvs.append(np.concatenate([dinv, np.zeros(NPAD - N, np.float32)]))
    structs = [build_structure(eis[t], ews[t], dinvs[t][:N]) for t in range(T)]

    # common per-section per-block L (max over cores; degree-sorted so tight)
    commonL = []
    for s in range(16):
        nb = max(st[s]["nblocks"] for st in structs)
        Lc = np.zeros(nb, np.int32)
        for st in structs:
            Ls = st[s]["Ls"]
            Lc[:len(Ls)] = np.maximum(Lc[:len(Ls)], Ls)
        commonL.append(Lc)

    tok_per_sec = [int(L.sum()) * P for L in commonL]
    blk_per_sec = [len(L) for L in commonL]
    tok_total = sum(tok_per_sec)

    per_gidx, per_w, per_sidx = [], [], []
    for c in range(T):
        g_all, w_all, s_all = [], [], []
        for s in range(16):
            sec = structs[c][s]
            Lc = commonL[s]
            nbm = len(Lc)
            own_nb = sec["nblocks"]
            # per-block dense grids in common shape
            for b in range(nbm):
                Lb = int(Lc[b])
                gi = np.zeros((Lb, P), np.int64)
                gw = np.zeros((Lb, P), np.float32)
                if b < own_nb:
                    sel = (sec["e_rank"] >= b * P) & (sec["e_rank"] < (b + 1) * P)
                    rr = sec["e_rank"][sel] - b * P
                    ss = sec["e_slot"][sel]
                    gi[ss, rr] = sec["e_src"][sel]
                    gw[ss, rr] = sec["e_w"][sel]
                g_all.append(gi.reshape(-1))
                w_all.append(gw.reshape(-1))
                if b < own_nb:
                    nd = sec["nodes"][b * P:(b + 1) * P].astype(np.int64)
                else:
                    nd = np.arange(QN, QN + P, dtype=np.int64)  # dummy block
                s_all.append(nd)
        per_gidx.append(np.concatenate(g_all).astype(np.int16))
        per_w.append(np.concatenate(w_all).astype(np.float32))
        per_sidx.append(np.concatenate(s_all).astype(np.int16))

    scat_total = sum(blk_per_sec) * P

    # blocked dinv layouts [128, NBLK]: col k = nodes k*128+p
    dinv_blk = [d.reshape(NBLK, P).T.copy() for d in dinvs]
    dinv2_blk = [(d * d).reshape(NBLK, P).T.copy() for d in dinvs]

    x_pad = np.zeros((NPAD, XD), np.float32)
    x_pad[:N] = x
    b1b = np.broadcast_to(b1, (P, H)).copy()
    b2b = np.broadcast_to(b2, (P, H)).copy()

    # ---------------- K1 program ----------------
    nc1 = bacc.Bacc(trn_type="TRN2", num_devices=NCORES, num_swdge_queues=4)
    x_d = nc1.dram_tensor("x", [NPAD, XD], F32, kind="ExternalInput")
    W1_d = nc1.dram_tensor("W1", [XD, H], F32, kind="ExternalInput")
    W2_d = nc1.dram_tensor("W2", [H, H], F32, kind="ExternalInput")
    b1_d = nc1.dram_tensor("b1b", [P, H], F32, kind="ExternalInput")
    b2_d = nc1.dram_tensor("b2b", [P, H], F32, kind="ExternalInput")
    dinv_d = nc1.dram_tensor("dinv_blk", [P, NBLK], F32, kind="ExternalInput")
    dinv2_d = nc1.dram_tensor("dinv2_blk", [P, NBLK], F32, kind="ExternalInput")
    gidx_d = nc1.dram_tensor("gidx", [P, tok_total // 16], I16, kind="ExternalInput")
    gw_d = nc1.dram_tensor("gw", [P, tok_total // P], F32, kind="ExternalInput")
    sidx_d = nc1.dram_tensor("sidx", [P, scat_total // 16], I16, kind="ExternalInput")
    tanhE_d = nc1.dram_tensor("tanhE", [NPAD, H], F32, kind="ExternalOutput")

    table1 = nc1.dram_tensor("table1", [NPAD, H], F32)
    table2 = nc1.dram_tensor("table2", [NPAD, H], F32)
    acc = [nc1.dram_tensor(f"acc{pr}", [4 * ACCR, H], F32) for pr in range(2)]

    with tile.TileContext(nc1) as tc:
        with (
            tc.tile_pool(name="const", bufs=1) as cpool,
            tc.tile_pool(name="work", bufs=3) as wpool,
            tc.tile_pool(name="gath", bufs=3) as gpool,
            tc.tile_pool(name="psum", bufs=2, space="PSUM") as ppool,
        ):
            ident = cpool.tile([P, P], F32)
            make_identity(nc1, ident[:])
            W1_t = cpool.tile([XD, H], F32)
            W2_t = cpool.tile([H, H], F32)
            b1_t = cpool.tile([P, H], F32)
            b2_t = cpool.tile([P, H], F32)
            dinv_t = cpool.tile([P, NBLK], F32)
            dinv2_t = cpool.tile([P, NBLK], F32)
            nc1.sync.dma_start(out=W1_t[:], in_=W1_d[:])
            nc1.sync.dma_start(out=W2_t[:], in_=W2_d[:])
            nc1.sync.dma_start(out=b1_t[:], in_=b1_d[:])
            nc1.sync.dma_start(out=b2_t[:], in_=b2_d[:])
            nc1.sync.dma_start(out=dinv_t[:], in_=dinv_d[:])
            nc1.sync.dma_start(out=dinv2_t[:], in_=dinv2_d[:])

            # zero accumulators
            zt = cpool.tile([P, 512], F32)
            nc1.gpsimd.memset(zt[:], 0.0)
            for pr in range(2):
                rows = 4 * ACCR
                a0 = 0
                while a0 < rows:
                    a1 = min(a0 + 1024, rows)
                    nc1.sync.dma_start(out=acc[pr][a0:a1, :],
                                       in_=zt[:, :(a1 - a0) // 2])
                    a0 = a1

            # stage A: table1 = dinv * (x @ W1)
            for k in range(NBLK):
                xb = wpool.tile([P, XD], F32, tag="xb")
                nc1.sync.dma_start(out=xb[:], in_=x_d[k * P:(k + 1) * P, :])
                xT_p = ppool.tile([P, P], F32, tag="pt")
                nc1.tensor.transpose(out=xT_p[:], in_=xb[:], identity=ident[:])
                xT = wpool.tile([P, P], F32, tag="xT")
                nc1.vector.tensor_copy(out=xT[:], in_=xT_p[:])
                mm = ppool.tile([P, H], F32, tag="mm")
                nc1.tensor.matmul(out=mm[:], lhsT=xT[:], rhs=W1_t[:],
                                  start=True, stop=True)
                tb = wpool.tile([P, H], F32, tag="tb")
                nc1.vector.tensor_scalar_mul(
                    out=tb[:], in0=mm[:], scalar1=dinv_t[:, k:k + 1])
                nc1.sync.dma_start(out=table1[k * P:(k + 1) * P, :], in_=tb[:])

            # the two props
            for pr in range(2):
                table = table1 if pr == 0 else table2
                go = 0   # token offset
                so = 0   # scatter token offset
                for s in range(16):
                    r, q = divmod(s, 4)
                    Lc = commonL[s]
                    tbl_slice = table[q * QN:(q + 1) * QN, :]
                    b = 0
                    while b < len(Lc):
                        L = int(Lc[b])
                        b2_ = b
                        while b2_ < len(Lc) and int(Lc[b2_]) == L:
                            b2_ += 1
                        nb = b2_ - b
                        if L == 0:
                            b = b2_
                            continue
                        # chunk over blocks (and slots if L>64)
                        gpc = max(1, 64 // L) if L <= 64 else 1
                        sub = min(L, 64)
                        bb = b
                        while bb < b2_:
                            nbb = min(gpc, b2_ - bb)
                            if L <= 64:
                                ncols = nbb * L
                                tok = ncols * P
                                gt = gpool.tile([P, 64, H], F32, tag="g")
                                gi_t = gpool.tile([P, 512], I16, tag="gi")
                                w_t = gpool.tile([P, 64], F32, tag="gwt")
                                nc1.sync.dma_start(
                                    out=gi_t[:, :tok // 16],
                                    in_=gidx_d[:, go // 16:(go + tok) // 16])
                                nc1.sync.dma_start(
                                    out=w_t[:, :ncols],
                                    in_=gw_d[:, go // P:(go + tok) // P])
                                nc1.gpsimd.dma_gather(
                                    out_ap=gt[:, :ncols, :], in_ap=tbl_slice,
                                    idxs_ap=gi_t[:, :tok // 16],
                                    num_idxs=tok, num_idxs_reg=tok,
                                    elem_size=H, single_packet=False,
                                    queue_num=(bb + s) % 4)
                                nc1.vector.tensor_tensor(
                                    out=gt[:, :ncols, :], in0=gt[:, :ncols, :],
                                    in1=w_t[:, :ncols].to_broadcast([P, ncols, H]),
                                    op=mybir.AluOpType.mult)
                                # tree reduce per g-group: view [P, nbb, L, H]
                                gv = gt[:, :ncols, :].rearrange(
                                    "p (g l) h -> p g l h", l=L)
                                width = L
                                while width > 1:
                                    half = width // 2
                                    nc1.vector.tensor_tensor(
                                        out=gv[:, :, :half, :],
                                        in0=gv[:, :, :half, :],
                                        in1=gv[:, :, width - half:width, :],
                                        op=mybir.AluOpType.add)
                                    width = width - half
                                # pack partials [P, nbb, H]
                                pk = gpool.tile([P, 64, H], F32, tag="pk")
                                nc1.vector.tensor_copy(
                                    out=pk[:, :nbb, :], in_=gv[:, :, 0, :])
                                stok_all = nbb * P
                                si_t = gpool.tile([P, 512], I16, tag="si")
                                nc1.sync.dma_start(
                                    out=si_t[:, :stok_all // 16],
                                    in_=sidx_d[:, so // 16:(so + stok_all) // 16])
                                for c0 in range(0, nbb, 8):
                                    c1 = min(c0 + 8, nbb)
                                    stok = (c1 - c0) * P
                                    nc1.gpsimd.dma_scatter_add(
                                        acc[pr][r * ACCR:(r + 1) * ACCR, :],
                                        pk[:, c0:c1, :],
                                        si_t[:, c0 * 8:c0 * 8 + stok // 16],
                                        stok, stok, H)
                                so += stok_all
                                go += tok
                            else:
                                # L > 64: one block, slot sub-chunks
                                first = True
                                for s0 in range(0, L, sub):
                                    s1 = min(s0 + sub, L)
                                    ncols = s1 - s0
                                    tok = ncols * P
                                    gt = gpool.tile([P, 64, H], F32, tag="g")
                                    gi_t = gpool.tile([P, 512], I16, tag="gi")
                                    w_t = gpool.tile([P, 64], F32, tag="gwt")
                                    nc1.sync.dma_start(
                                        out=gi_t[:, :tok // 16],
                                        in_=gidx_d[:, go // 16:(go + tok) // 16])
                                    nc1.sync.dma_start(
                                        out=w_t[:, :ncols],
                                        in_=gw_d[:, go // P:(go + tok) // P])
                                    nc1.gpsimd.dma_gather(
                                        out_ap=gt[:, :ncols, :], in_ap=tbl_slice,
                                        idxs_ap=gi_t[:, :tok // 16],
                                        num_idxs=tok, num_idxs_reg=tok,
                                        elem_size=H, single_packet=False,
                                        queue_num=(bb + s0 + s) % 4)
                                    nc1.vector.tensor_tensor(
                                        out=gt[:, :ncols, :], in0=gt[:, :ncols, :],
                                        in1=w_t[:, :ncols].to_broadcast([P, ncols, H]),
                                        op=mybir.AluOpType.mult)
                                    width = ncols
                                    gv = gt[:, :ncols, :].rearrange(
                                        "p (g l) h -> p g l h", l=ncols)
                                    while width > 1:
                                        half = width // 2
                                        nc1.vector.tensor_tensor(
                                            out=gv[:, :, :half, :],
                                            in0=gv[:, :, :half, :],
                                            in1=gv[:, :, width - half:width, :],
                                            op=mybir.AluOpType.add)
                                        width = width - half
                                    pk = gpool.tile([P, 64, H], F32, tag="pk")
                                    nc1.vector.tensor_copy(
                                        out=pk[:, :1, :], in_=gv[:, :, 0, :])
                                    si_t = gpool.tile([P, 64], I16, tag="si")
                                    nc1.sync.dma_start(
                                        out=si_t[:, :P // 16],
                                        in_=sidx_d[:, so // 16:(so + P) // 16])
                                    nc1.gpsimd.dma_scatter_add(
                                        acc[pr][r * ACCR:(r + 1) * ACCR, :],
                                        pk[:, :1, :], si_t[:, :P // 16],
                                        P, P, H)
                                    go += tok
                                    first = False
                                so += P
                            bb += nbb if L <= 64 else 1
                        b = b2_

                # finish pass
                for k in range(NBLK):
                    r = (k * P) // QN
                    lrow = (k * P) % QN
                    ab = wpool.tile([P, H], F32, tag="ab")
                    nc1.sync.dma_start(
                        out=ab[:], in_=acc[pr][r * ACCR + lrow:r * ACCR + lrow + P, :])
                    tb = wpool.tile([P, H], F32, tag="tb2")
                    nc1.sync.dma_start(out=tb[:], in_=table[k * P:(k + 1) * P, :])
                    init = wpool.tile([P, H], F32, tag="init")
                    nc1.vector.tensor_scalar_mul(
                        out=init[:], in0=tb[:], scalar1=dinv_t[:, k:k + 1])
                    nc1.vector.tensor_tensor(out=ab[:], in0=ab[:], in1=init[:],
                                             op=mybir.AluOpType.add)
                    if pr == 0:
                        # h1 = relu(ab + b1); table2 = dinv * h1
                        nc1.vector.tensor_tensor(out=ab[:], in0=ab[:], in1=b1_t[:],
                                                 op=mybir.AluOpType.add)
                        nc1.vector.tensor_scalar_max(out=ab[:], in0=ab[:], scalar1=0.0)
                        ot = wpool.tile([P, H], F32, tag="ot")
                        nc1.vector.tensor_scalar_mul(
                            out=ot[:], in0=ab[:], scalar1=dinv_t[:, k:k + 1])
                        nc1.sync.dma_start(out=table2[k * P:(k + 1) * P, :], in_=ot[:])
                    else:
                        # embeds = ab @ W2 + b2 ; tanhE = tanh(embeds)
                        tp = ppool.tile([P, P], F32, tag="pt")
                        nc1.tensor.transpose(out=tp[:H, :], in_=ab[:],
                                             identity=ident[:])
                        abT = wpool.tile([P, P], F32, tag="abT")
                        nc1.vector.tensor_copy(out=abT[:H, :], in_=tp[:H, :])
                        mm = ppool.tile([P, H], F32, tag="mm")
                        nc1.tensor.matmul(out=mm[:], lhsT=abT[:H, :], rhs=W2_t[:],
                                          start=True, stop=True)
                        eb = wpool.tile([P, H], F32, tag="eb")
                        nc1.vector.tensor_tensor(out=eb[:], in0=mm[:], in1=b2_t[:],
                                                 op=mybir.AluOpType.add)
                        th = wpool.tile([P, H], F32, tag="th")
                        nc1.scalar.activation(
                            out=th[:], in_=eb[:],
                            func=mybir.ActivationFunctionType.Tanh)
                        nc1.sync.dma_start(out=tanhE_d[k * P:(k + 1) * P, :],
                                           in_=th[:])
    nc1.compile()

    in_maps1 = []
    for c in range(NCORES):
        in_maps1.append({
            "x": x_pad, "W1": W1, "W2": W2, "b1b": b1b, "b2b": b2b,
            "dinv_blk": dinv_blk[c], "dinv2_blk": dinv2_blk[c],
            "gidx": wrap16(per_gidx[c]),
            "gw": per_w[c].reshape(-1, P).T.copy(),
            "sidx": wrap16(per_sidx[c]),
            "tanhE": np.zeros((NPAD, H), np.float32),
        })
    res1 = run_bass_kernel_spmd(nc1, in_maps1, core_ids=list(range(NCORES)))
    tanhE = np.stack([res1.results[c]["tanhE"] for c in range(NCORES)])  # [T,NPAD,H]

    # ---------------- K2: GRU + head, node-sharded ----------------
    WihT = Wih.T.copy()    # [H, 3H]
    WhhT = Whh.T.copy()
    WlinT = Wlin.T.copy()  # [H, Z]
    bihb = np.broadcast_to(bih, (P, 3 * H)).copy()
    bhhb = np.broadcast_to(bhh, (P, 3 * H)).copy()
    blinb = np.broadcast_to(blin, (P, Z)).copy()

    nc2 = bacc.Bacc(trn_type="TRN2", num_devices=NCORES, num_swdge_queues=1)
    xs_d = nc2.dram_tensor("xs", [T, NSH, H], F32, kind="ExternalInput")
    WihT_d = nc2.dram_tensor("WihT", [H, 3 * H], F32, kind="ExternalInput")
    WhhT_d = nc2.dram_tensor("WhhT", [H, 3 * H], F32, kind="ExternalInput")
    WlinT_d = nc2.dram_tensor("WlinT", [H, Z], F32, kind="ExternalInput")
    bih_d = nc2.dram_tensor("bihb", [P, 3 * H], F32, kind="ExternalInput")
    bhh_d = nc2.dram_tensor("bhhb", [P, 3 * H], F32, kind="ExternalInput")
    blin_d = nc2.dram_tensor("blinb", [P, Z], F32, kind="ExternalInput")
    ys_d = nc2.dram_tensor("ys", [T, NSH, Z], F32, kind="ExternalOutput")

    with tile.TileContext(nc2) as tc:
        with (
            tc.tile_pool(name="const", bufs=1) as cpool,
            tc.tile_pool(name="state", bufs=1) as spool,
            tc.tile_pool(name="work", bufs=3) as wpool,
            tc.tile_pool(name="psum", bufs=2, space="PSUM") as ppool,
        ):
            ident = cpool.tile([P, P], F32)
            make_identity(nc2, ident[:])
            WihT_t = cpool.tile([H, 3 * H], F32)
            WhhT_t = cpool.tile([H, 3 * H], F32)
            WlinT_t = cpool.tile([H, Z], F32)
            bih_t = cpool.tile([P, 3 * H], F32)
            bhh_t = cpool.tile([P, 3 * H], F32)
            blin_t = cpool.tile([P, Z], F32)
            for tt, dd in ((WihT_t, WihT_d), (WhhT_t, WhhT_d), (WlinT_t, WlinT_d),
                           (bih_t, bih_d), (bhh_t, bhh_d), (blin_t, blin_d)):
                nc2.sync.dma_start(out=tt[:], in_=dd[:])

            JC2 = JC // 2
            NSH2 = NSH // 2
            for half in range(2):
                h_t = spool.tile([P, JC2, H], F32, tag="h")
                nc2.gpsimd.memset(h_t[:], 0.0)
                for t in range(T):
                    xs_t = spool.tile([P, JC2, H], F32, tag="xs")
                    xv = xs_d[t][half * NSH2:(half + 1) * NSH2, :].rearrange(
                        "(p j) h -> p j h", j=JC2)
                    nc2.sync.dma_start(out=xs_t[:], in_=xv)
                    gi_t = spool.tile([P, JC2, 3 * H], F32, tag="gi")
                    gh_t = spool.tile([P, JC2, 3 * H], F32, tag="gh")
                    for j in range(JC2):
                        for which in range(2):
                            srcT = xs_t if which == 0 else h_t
                            dstT = gi_t if which == 0 else gh_t
                            tp = ppool.tile([P, P], F32, tag="pt")
                            nc2.tensor.transpose(out=tp[:H, :], in_=srcT[:, j, :],
                                                 identity=ident[:])
                            sT = wpool.tile([H, P], F32, tag="sT")
                            nc2.vector.tensor_copy(out=sT[:], in_=tp[:H, :])
                            mm = ppool.tile([P, 3 * H], F32, tag="mm")
                            WT = WihT_t if which == 0 else WhhT_t
                            nc2.tensor.matmul(out=mm[:], lhsT=sT[:], rhs=WT[:],
                                              start=True, stop=True)
                            bT = bih_t if which == 0 else bhh_t
                            nc2.vector.tensor_tensor(out=dstT[:, j, :], in0=mm[:],
                                                     in1=bT[:],
                                                     op=mybir.AluOpType.add)
                    rz = spool.tile([P, JC2, 2 * H], F32, tag="rz")
                    nc2.vector.tensor_tensor(out=rz[:], in0=gi_t[:, :, :2 * H],
                                             in1=gh_t[:, :, :2 * H],
                                             op=mybir.AluOpType.add)
                    nc2.scalar.activation(out=rz[:], in_=rz[:],
                                          func=mybir.ActivationFunctionType.Sigmoid)
                    nn_t = spool.tile([P, JC2, H], F32, tag="nn")
                    nc2.vector.tensor_tensor(out=nn_t[:], in0=rz[:, :, :H],
                                             in1=gh_t[:, :, 2 * H:],
                                             op=mybir.AluOpType.mult)
                    nc2.vector.tensor_tensor(out=nn_t[:], in0=nn_t[:],
                                             in1=gi_t[:, :, 2 * H:],
                                             op=mybir.AluOpType.add)
                    nc2.scalar.activation(out=nn_t[:], in_=nn_t[:],
                                          func=mybir.ActivationFunctionType.Tanh)
                    dz = spool.tile([P, JC2, H], F32, tag="dz")
                    nc2.vector.tensor_tensor(out=dz[:], in0=h_t[:], in1=nn_t[:],
                                             op=mybir.AluOpType.subtract)
                    nc2.vector.tensor_tensor(out=dz[:], in0=dz[:], in1=rz[:, :, H:],
                                             op=mybir.AluOpType.mult)
                    nc2.vector.tensor_tensor(out=h_t[:], in0=nn_t[:], in1=dz[:],
                                             op=mybir.AluOpType.add)
                    ys_t = spool.tile([P, JC2, Z], F32, tag="ys")
                    for j in range(JC2):
                        tp = ppool.tile([P, P], F32, tag="pt")
                        nc2.tensor.transpose(out=tp[:H, :], in_=h_t[:, j, :],
                                             identity=ident[:])
                        sT = wpool.tile([H, P], F32, tag="sT")
                        nc2.vector.tensor_copy(out=sT[:], in_=tp[:H, :])
                        mm = ppool.tile([P, Z], F32, tag="mmz")
                        nc2.tensor.matmul(out=mm[:], lhsT=sT[:], rhs=WlinT_t[:],
                                          start=True, stop=True)
                        nc2.vector.tensor_tensor(out=ys_t[:, j, :], in0=mm[:],
                                                 in1=blin_t[:],
                                                 op=mybir.AluOpType.add)
                    yv = ys_d[t][half * NSH2:(half + 1) * NSH2, :].rearrange(
                        "(p j) z -> p j z", j=JC2)
                    nc2.sync.dma_start(out=yv, in_=ys_t[:])
    nc2.compile()

    in_maps2 = []
    for c in range(NCORES):
        xs = np.ascontiguousarray(tanhE[:, c * NSH:(c + 1) * NSH, :])
        in_maps2.append({
            "xs": xs, "WihT": WihT, "WhhT": WhhT, "WlinT": WlinT,
            "bihb": bihb, "bhhb": bhhb, "blinb": blinb,
            "ys": np.zeros((T, NSH, Z), np.float32),
        })
    res2 = run_bass_kernel_spmd(nc2, in_maps2, core_ids=list(range(NCORES)))
    out = np.concatenate([res2.results[c]["ys"] for c in range(NCORES)], axis=1)
    return np.ascontiguousarray(out[:, :N, :])



# revision 11
# speedup vs baseline: 1.2040x; 1.2040x over previous
"""EulerGCN on 8 trn2 NeuronCores — single SPMD launch.

Core t owns snapshot t for the GCN encode: 2 GCN props via ELL gathers +
DVE tree reduce + scatter-add into natural-order DRAM accumulators
(self-loops folded in as ordinary edge tokens; both props share one token
stream since the adjacency is identical). finish1 is a pure streaming
DVE pass; finish2 applies W2 per 128-node block via PE transpose+matmul
and emits tanh(emb)^T feature-major. An in-NEFF AllToAll reshards
feature-major slabs to node-parallel, then a transposed GRU + linear
head run in the same NEFF. Host does integer layout (edge grouping,
degree sort, token grids), GCN normalization, x@W1, and the final
output transpose.
"""

import sys
import time
import numpy as np
import ml_dtypes
import concourse.bass as bass
import concourse.bacc as bacc
import concourse.mybir as mybir
import concourse.tile as tile
from concourse.bass_utils import run_bass_kernel_spmd
from concourse.masks import make_identity

P = 128
NCORES = 8
N = 100000
NPAD = 100352           # 784 blocks of 128
QN = NPAD // 4          # 25088
QBLK = QN // P          # 196
NBLK = NPAD // P        # 784
T = 8
XD = 128
H = 64
Z = 32
NSH = NPAD // NCORES    # 12544
GCH = 448               # GRU chunk cols (28 * 448 = NSH)
NGCH = NSH // GCH
F32 = mybir.dt.float32
BF16 = mybir.dt.bfloat16
I16 = mybir.dt.int16
BF = ml_dtypes.bfloat16

PERF = {}


def _tick(label, t0):
    dt = time.time() - t0
    PERF[label] = PERF.get(label, 0.0) + dt
    print(f"[kernel] {label}: {dt:.2f}s", file=sys.stderr, flush=True)
    return time.time()


def wrap16(a):
    return np.ascontiguousarray(a.reshape(-1, 16).T)


def _prep_core(args):
    eis_c0, eis_c1, ews_c = args
    src = eis_c0.astype(np.int64)
    dst = eis_c1.astype(np.int64)
    w = ews_c.astype(np.float32)
    deg = np.bincount(dst, weights=w, minlength=N).astype(np.float32) + 1.0
    dinv = 1.0 / np.sqrt(deg)                                # [N]
    loops = np.arange(N, dtype=np.int64)
    src = np.concatenate([src, loops])
    dst = np.concatenate([dst, loops])
    wd = np.concatenate([w, np.ones(N, np.float32)]) * dinv[dst]
    sec = (dst // QN) * 4 + (src // QN)
    eo = np.argsort(sec, kind="stable")
    sec_counts = np.bincount(sec, minlength=16)
    src, dst, wd = src[eo], dst[eo], wd[eo]
    bounds = np.concatenate([[0], np.cumsum(sec_counts)])
    secs = []
    for s in range(16):
        r, q = divmod(s, 4)
        sl = slice(bounds[s], bounds[s + 1])
        dl = dst[sl] - r * QN
        s_l = src[sl] - q * QN
        w_l = wd[sl]
        cnt = np.bincount(dl, minlength=QN)
        order = np.argsort(-cnt, kind="stable")              # full QN perm
        rank_of = np.empty(QN, np.int64)
        rank_of[order] = np.arange(QN)
        er = rank_of[dl]
        ei2 = np.argsort(er, kind="stable")
        er_s = er[ei2]
        slot = np.arange(er_s.size) - np.searchsorted(er_s, er_s)
        Ls = cnt[order].reshape(QBLK, P).max(axis=1).astype(np.int64)
        secs.append(dict(er=er_s.astype(np.int32), slot=slot.astype(np.int16),
                         src=s_l[ei2].astype(np.int16), w=w_l[ei2],
                         order=order.astype(np.int32), Ls=Ls))
    return dict(dinv=dinv, secs=secs)


def build_host(x, eis, ews, W1):
    """Per-core tables and shared-shape token grids."""
    xw1 = x.astype(np.float32) @ W1.astype(np.float32)
    percore = [_prep_core((eis[c, 0], eis[c, 1], ews[c]))
               for c in range(T)]

    # common per-section block L (max over cores)
    commonL, nbs = [], []
    for s in range(16):
        Lc = np.zeros(QBLK, np.int64)
        for pc in percore:
            Lc = np.maximum(Lc, pc["secs"][s]["Ls"])
        nz = np.nonzero(Lc)[0]
        nb = int(nz[-1]) + 1 if nz.size else 1
        commonL.append(Lc[:nb])
        nbs.append(nb)
    sec_tok = [int(L.sum()) * P for L in commonL]
    sec_scat = [nb * P for nb in nbs]
    tok_total = sum(sec_tok)
    scat_total = sum(sec_scat)

    offs = [np.concatenate([[0], np.cumsum(Lc)]) * P for Lc in commonL]

    def _streams_core(c):
        g_all = np.zeros(tok_total, np.int16)
        w_all = np.zeros(tok_total, np.float32)
        s_all = np.empty(scat_total, np.int16)
        go = so = 0
        for s in range(16):
            ssec = percore[c]["secs"][s]
            off = offs[s]
            er, slot = ssec["er"], ssec["slot"]
            pos = off[er >> 7] + slot * P + (er & 127)
            g_all[go + pos] = ssec["src"].astype(np.int16)
            w_all[go + pos] = ssec["w"]
            s_all[so:so + sec_scat[s]] = ssec["order"][:sec_scat[s]].astype(np.int16)
            go += sec_tok[s]
            so += sec_scat[s]
        return dict(gidx=g_all, gw=w_all, sidx=s_all)

    streams = [_streams_core(c) for c in range(T)]

    xw1p = np.zeros((NPAD, H), np.float32)
    xw1p[:N] = xw1
    xw1bf = xw1p.astype(BF)
    tables = []
    for c in range(T):
        dpad = np.zeros(NPAD, np.float32)
        dpad[:N] = percore[c]["dinv"]
        tables.append(dict(dinv_blk=dpad.reshape(NBLK, P).T.copy()))
    return dict(commonL=commonL, nbs=nbs, sec_tok=sec_tok, sec_scat=sec_scat,
                tok_total=tok_total, scat_total=scat_total,
                streams=streams, tables=tables, xw1bf=xw1bf)


def build_program(hp):
    commonL = hp["commonL"]
    sec_tok = hp["sec_tok"]
    sec_scat = hp["sec_scat"]
    tok_total = hp["tok_total"]
    scat_total = hp["scat_total"]
    max_tok = max(sec_tok)
    max_scat = max(sec_scat)

    nc = bacc.Bacc(trn_type="TRN2", num_devices=NCORES, num_swdge_queues=4)
    t1bf_d = nc.dram_tensor("xw1bf", [NPAD, H], BF16, kind="ExternalInput")
    gidx_d = nc.dram_tensor("gidx16", [16, tok_total // 16], I16, kind="ExternalInput")
    gw_d = nc.dram_tensor("gw128", [P, tok_total // P], BF16, kind="ExternalInput")
    sidx_d = nc.dram_tensor("sidx16", [16, scat_total // 16], I16, kind="ExternalInput")
    dinv_d = nc.dram_tensor("dinv_blk", [P, NBLK], F32, kind="ExternalInput")
    b1b_d = nc.dram_tensor("b1b", [P, H], F32, kind="ExternalInput")
    b2c_d = nc.dram_tensor("b2c", [H, 1], F32, kind="ExternalInput")
    W2_d = nc.dram_tensor("W2", [H, H], F32, kind="ExternalInput")
    wihT_d = nc.dram_tensor("wihT", [H, 3 * H], BF16, kind="ExternalInput")
    whhT_d = nc.dram_tensor("whhT", [H, 3 * H], BF16, kind="ExternalInput")
    wlinT_d = nc.dram_tensor("wlinT", [H, Z], BF16, kind="ExternalInput")
    br_d = nc.dram_tensor("br", [H, 1], F32, kind="ExternalInput")
    bz_d = nc.dram_tensor("bz", [H, 1], F32, kind="ExternalInput")
    bin_d = nc.dram_tensor("bin", [H, 1], F32, kind="ExternalInput")
    bhn_d = nc.dram_tensor("bhn", [H, 1], F32, kind="ExternalInput")
    blin_d = nc.dram_tensor("blin", [Z, 1], F32, kind="ExternalInput")
    ysT_d = nc.dram_tensor("ysT", [T, Z, NSH], BF16, kind="ExternalOutput")

    table1 = nc.dram_tensor("table1", [NPAD, H], F32)
    table2 = nc.dram_tensor("table2", [NPAD, H], F32)
    acc = [nc.dram_tensor(f"acc{pr}", [NPAD, H], F32) for pr in range(2)]

    with tile.TileContext(nc) as tc:
        with tc.tile_pool(name="const", bufs=1) as cpool, \
             tc.tile_pool(name="dram", bufs=1, space="DRAM") as dpool:
            ident = cpool.tile([P, P], F32)
            make_identity(nc, ident[:])
            dinv_t = cpool.tile([P, NBLK], F32)
            b1_t = cpool.tile([P, H], F32)
            b2c_t = cpool.tile([H, 1], F32)
            W2_t = cpool.tile([H, H], F32)
            wih_t = cpool.tile([H, 3 * H], BF16)
            whh_t = cpool.tile([H, 3 * H], BF16)
            wlin_t = cpool.tile([H, Z], BF16)
            br_t = cpool.tile([H, 1], F32)
            bz_t = cpool.tile([H, 1], F32)
            bin_t = cpool.tile([H, 1], F32)
            bhn_t = cpool.tile([H, 1], F32)
            blin_t = cpool.tile([Z, 1], F32)
            for tt, dd in ((dinv_t, dinv_d), (b1_t, b1b_d), (b2c_t, b2c_d),
                           (W2_t, W2_d), (wih_t, wihT_d), (whh_t, whhT_d),
                           (wlin_t, wlinT_d), (br_t, br_d), (bz_t, bz_d),
                           (bin_t, bin_d), (bhn_t, bhn_d), (blin_t, blin_d)):
                nc.sync.dma_start(out=tt[:], in_=dd[:])

            cc_in = dpool.tile([NCORES * H, NSH], BF16)
            cc_out = dpool.tile([NCORES * H, NSH], BF16)

            # zero accumulators (32 x 0.8MB DMAs)
            zt = cpool.tile([P, 1568], F32)
            nc.gpsimd.memset(zt[:], 0.0)
            for pr in range(2):
                for a0 in range(0, NPAD, 3136):
                    nc.sync.dma_start(out=acc[pr][a0:a0 + 3136, :],
                                      in_=zt[:])

            # expand table1 bf16 -> f32 (8 chunks of 98 blocks)
            with tc.tile_pool(name="exp", bufs=2) as epool:
                for k in range(0, NBLK, 98):
                    src = t1bf_d[k * P:(k + 98) * P, :].rearrange(
                        "(j p) h -> p j h", p=P)
                    tb = epool.tile([P, 98, H], BF16, tag="tbf")
                    nc.sync.dma_start(out=tb[:], in_=src)
                    tf = epool.tile([P, 98, H], F32, tag="tf32")
                    nc.vector.tensor_copy(out=tf[:], in_=tb[:])
                    nc.vector.tensor_tensor(
                        out=tf[:], in0=tf[:],
                        in1=dinv_t[:, k:k + 98].unsqueeze(-1)
                            .broadcast_to([P, 98, H]),
                        op=mybir.AluOpType.mult)
                    nc.sync.dma_start(
                        out=table1[k * P:(k + 98) * P, :].rearrange(
                            "(j p) h -> p j h", p=P),
                        in_=tf[:])

            # ---- the two props ----
            with tc.tile_pool(name="sec", bufs=2) as spool, \
                 tc.tile_pool(name="gath", bufs=3) as gpool:
                qcount = 0
                for pr in range(2):
                    table = table1 if pr == 0 else table2
                    go = so = 0
                    for s in range(16):
                        r, q = divmod(s, 4)
                        Lc = commonL[s]
                        stok, ssc = sec_tok[s], sec_scat[s]
                        if stok == 0:
                            go += stok
                            so += ssc
                            continue
                        gi_b = spool.tile([P, max_tok // 16], I16, tag="gi")
                        si_b = spool.tile([P, max_scat // 16], I16, tag="si")
                        for k in range(8):
                            nc.sync.dma_start(
                                out=gi_b[16 * k:16 * k + 16, :stok // 16],
                                in_=gidx_d[:, go // 16:(go + stok) // 16])
                            nc.sync.dma_start(
                                out=si_b[16 * k:16 * k + 16, :ssc // 16],
                                in_=sidx_d[:, so // 16:(so + ssc) // 16])
                        wbf = spool.tile([P, max_tok // P], BF16, tag="wbf")
                        nc.sync.dma_start(out=wbf[:, :stok // P],
                                          in_=gw_d[:, go // P:(go + stok) // P])
                        w_b = spool.tile([P, max_tok // P], F32, tag="wf")
                        nc.vector.tensor_copy(out=w_b[:, :stok // P],
                                              in_=wbf[:, :stok // P])

                        tbl = table[q * QN:(q + 1) * QN, :]
                        accr = acc[pr][r * QN:(r + 1) * QN, :]
                        lgo = lso = 0   # local token / scatter offsets
                        b = 0
                        while b < len(Lc):
                            L = int(Lc[b])
                            b2 = b
                            while b2 < len(Lc) and int(Lc[b2]) == L:
                                b2 += 1
                            if L == 0:
                                b = b2
                                continue
                            assert L <= 64, L
                            gpc = max(1, 64 // L)
                            bb = b
                            while bb < b2:
                                nbb = min(gpc, b2 - bb)
                                ncols = nbb * L
                                tok = ncols * P
                                stk = nbb * P
                                pk = gpool.tile([P, 64, H], F32, tag="pk")
                                if L == 1:
                                    gt = gpool.tile([P, 64, H], F32, tag="g")
                                    nc.gpsimd.dma_gather(
                                        out_ap=gt[:, :ncols, :], in_ap=tbl,
                                        idxs_ap=gi_b[:, lgo // 16:(lgo + tok) // 16],
                                        num_idxs=tok, num_idxs_reg=tok,
                                        elem_size=H, single_packet=False,
                                        queue_num=qcount % 4)
                                    nc.vector.tensor_tensor(
                                        out=pk[:, :ncols, :], in0=gt[:, :ncols, :],
                                        in1=w_b[:, lgo // P:lgo // P + ncols]
                                            .unsqueeze(-1)
                                            .broadcast_to([P, ncols, H]),
                                        op=mybir.AluOpType.mult)
                                else:
                                    gt = gpool.tile([P, 64, H], F32, tag="g")
                                    nc.gpsimd.dma_gather(
                                        out_ap=gt[:, :ncols, :], in_ap=tbl,
                                        idxs_ap=gi_b[:, lgo // 16:(lgo + tok) // 16],
                                        num_idxs=tok, num_idxs_reg=tok,
                                        elem_size=H, single_packet=False,
                                        queue_num=qcount % 4)
                                    nc.vector.tensor_tensor(
                                        out=gt[:, :ncols, :], in0=gt[:, :ncols, :],
                                        in1=w_b[:, lgo // P:lgo // P + ncols]
                                            .unsqueeze(-1)
                                            .broadcast_to([P, ncols, H]),
                                        op=mybir.AluOpType.mult)
                                    gv = gt[:, :ncols, :].rearrange(
                                        "p (g l) h -> p g l h", l=L)
                                    width = L
                                    while width > 2:
                                        half = width // 2
                                        nc.vector.tensor_tensor(
                                            out=gv[:, :, :half, :],
                                            in0=gv[:, :, :half, :],
                                            in1=gv[:, :, width - half:width, :],
                                            op=mybir.AluOpType.add)
                                        width -= half
                                    if width == 2:
                                        nc.vector.tensor_tensor(
                                            out=pk[:, :nbb, :],
                                            in0=gv[:, :, 0, :], in1=gv[:, :, 1, :],
                                            op=mybir.AluOpType.add)
                                    else:
                                        nc.vector.tensor_copy(
                                            out=pk[:, :nbb, :], in_=gv[:, :, 0, :])
                                nc.gpsimd.dma_scatter_add(
                                    accr, pk[:, :nbb, :],
                                    si_b[:, lso // 16:(lso + stk) // 16],
                                    stk, stk, H, queue_num=qcount % 4)
                                qcount += 1
                                lgo += tok
                                lso += stk
                                bb += nbb
                            b = b2
                        go += stok
                        so += ssc

                    # ---- finish pass ----
                    if pr == 0:
                        with tc.tile_pool(name="fin", bufs=2) as fpool:
                            for k in range(0, NBLK, 49):
                                av = fpool.tile([P, 49, H], F32, tag="av")
                                nc.sync.dma_start(
                                    out=av[:],
                                    in_=acc[0][k * P:(k + 49) * P, :].rearrange(
                                        "(j p) h -> p j h", p=P))
                                nc.vector.tensor_tensor(
                                    out=av[:], in0=av[:],
                                    in1=b1_t[:].unsqueeze(1)
                                        .broadcast_to([P, 49, H]),
                                    op=mybir.AluOpType.add)
                                nc.vector.tensor_scalar_max(
                                    out=av[:], in0=av[:], scalar1=0.0)
                                nc.vector.tensor_tensor(
                                    out=av[:], in0=av[:],
                                    in1=dinv_t[:, k:k + 49].unsqueeze(-1)
                                        .broadcast_to([P, 49, H]),
                                    op=mybir.AluOpType.mult)
                                nc.sync.dma_start(
                                    out=table2[k * P:(k + 49) * P, :].rearrange(
                                        "(j p) h -> p j h", p=P),
                                    in_=av[:])

            # ---- finish2: W2, bias, tanh, transpose to feature-major ----
            with tc.tile_pool(name="f2", bufs=3) as f2pool, \
                 tc.tile_pool(name="f2p", bufs=4, space="PSUM") as f2ps:
                for j in range(NCORES):          # peer slab
                    for c0 in range(0, 98, 8):
                        nb2 = min(8, 98 - c0)
                        k0 = j * 98 + c0
                        av2 = f2pool.tile([P, 8, H], F32, tag="av2")
                        nc.sync.dma_start(
                            out=av2[:, :nb2, :],
                            in_=acc[1][k0 * P:(k0 + nb2) * P, :].rearrange(
                                "(j p) h -> p j h", p=P))
                        for g0 in range(0, nb2, 4):
                            ng = min(4, nb2 - g0)
                            eg = f2pool.tile([H, 4 * P], BF16, tag="eg")
                            for bi in range(ng):
                                pt = f2ps.tile([H, P], F32, tag="pt")
                                nc.tensor.transpose(
                                    out=pt[:], in_=av2[:, g0 + bi, :],
                                    identity=ident[:])
                                abT = f2pool.tile([H, P], F32, tag="abT")
                                nc.vector.tensor_copy(out=abT[:], in_=pt[:])
                                mm = f2ps.tile([H, P], F32, tag="mm")
                                nc.tensor.matmul(out=mm[:], lhsT=W2_t[:],
                                                 rhs=abT[:], start=True, stop=True)
                                nc.scalar.activation(
                                    out=eg[:, bi * P:(bi + 1) * P], in_=mm[:],
                                    func=mybir.ActivationFunctionType.Tanh,
                                    bias=b2c_t[:])
                            cz = (c0 + g0) * P
                            nc.sync.dma_start(
                                out=cc_in[j * H:(j + 1) * H, cz:cz + ng * P],
                                in_=eg[:, :ng * P])

            # ---- AllToAll reshard ----
            nc.gpsimd.collective_compute(
                "AllToAll", mybir.AluOpType.bypass,
                replica_groups=[list(range(NCORES))],
                ins=[cc_in[:]], outs=[cc_out[:]])

            # ---- GRU + head (transposed layout) ----
            with tc.tile_pool(name="gs", bufs=1) as gspool, \
                 tc.tile_pool(name="gx", bufs=2) as gxpool, \
                 tc.tile_pool(name="gw", bufs=2) as gwpool, \
                 tc.tile_pool(name="gp", bufs=2, space="PSUM") as gppool, \
                 tc.tile_pool(name="gp1", bufs=1, space="PSUM") as gppool1:
                h32 = gspool.tile([H, NSH], F32)
                nc.gpsimd.memset(h32[:], 0.0)
                for t in range(T):
                    xsT = gxpool.tile([H, NSH], BF16, tag="xs")
                    nc.sync.dma_start(out=xsT[:],
                                      in_=cc_out[t * H:(t + 1) * H, :])
                    y_t = gxpool.tile([Z, NSH], BF16, tag="y")
                    for i in range(NGCH):
                        sl = slice(i * GCH, (i + 1) * GCH)
                        hb = gwpool.tile([H, GCH], BF16, tag="hb")
                        nc.vector.tensor_copy(out=hb[:], in_=h32[:, sl])
                        mm_r = gppool.tile([H, GCH], F32, tag="mr")
                        nc.tensor.matmul(out=mm_r[:], lhsT=wih_t[:, :H],
                                         rhs=xsT[:, sl], start=True, stop=False)
                        nc.tensor.matmul(out=mm_r[:], lhsT=whh_t[:, :H],
                                         rhs=hb[:], start=False, stop=True)
                        mm_z = gppool.tile([H, GCH], F32, tag="mz")
                        nc.tensor.matmul(out=mm_z[:], lhsT=wih_t[:, H:P],
                                         rhs=xsT[:, sl], start=True, stop=False)
                        nc.tensor.matmul(out=mm_z[:], lhsT=whh_t[:, H:P],
                                         rhs=hb[:], start=False, stop=True)
                        r_sb = gwpool.tile([H, GCH], F32, tag="r")
                        nc.scalar.activation(
                            out=r_sb[:], in_=mm_r[:],
                            func=mybir.ActivationFunctionType.Sigmoid,
                            bias=br_t[:])
                        z_sb = gwpool.tile([H, GCH], F32, tag="z")
                        nc.scalar.activation(
                            out=z_sb[:], in_=mm_z[:],
                            func=mybir.ActivationFunctionType.Sigmoid,
                            bias=bz_t[:])
                        mm_hn = gppool1.tile([H, GCH], F32, tag="mhn")
                        nc.tensor.matmul(out=mm_hn[:], lhsT=whh_t[:, P:],
                                         rhs=hb[:], start=True, stop=True)
                        rn = gwpool.tile([H, GCH], F32, tag="rn")
                        nc.vector.tensor_scalar_add(
                            out=rn[:], in0=mm_hn[:], scalar1=bhn_t[:])
                        nc.vector.tensor_tensor(
                            out=rn[:], in0=rn[:], in1=r_sb[:],
                            op=mybir.AluOpType.mult)
                        mm_in = gppool1.tile([H, GCH], F32, tag="min")
                        nc.tensor.matmul(out=mm_in[:], lhsT=wih_t[:, P:],
                                         rhs=xsT[:, sl], start=True, stop=True)
                        npre = gwpool.tile([H, GCH], F32, tag="npre")
                        nc.vector.tensor_tensor(
                            out=npre[:], in0=mm_in[:], in1=rn[:],
                            op=mybir.AluOpType.add)
                        n_sb = gwpool.tile([H, GCH], F32, tag="nsb")
                        nc.scalar.activation(
                            out=n_sb[:], in_=npre[:],
                            func=mybir.ActivationFunctionType.Tanh,
                            bias=bin_t[:])
                        d = gwpool.tile([H, GCH], F32, tag="d")
                        nc.vector.tensor_tensor(
                            out=d[:], in0=h32[:, sl], in1=n_sb[:],
                            op=mybir.AluOpType.subtract)
                        nc.vector.tensor_tensor(
                            out=d[:], in0=d[:], in1=z_sb[:],
                            op=mybir.AluOpType.mult)
                        nc.vector.tensor_tensor(
                            out=h32[:, sl], in0=n_sb[:], in1=d[:],
                            op=mybir.AluOpType.add)
                        hb2 = gwpool.tile([H, GCH], BF16, tag="hb2")
                        nc.vector.tensor_copy(out=hb2[:], in_=h32[:, sl])
                        mm_y = gppool.tile([Z, GCH], F32, tag="my")
                        nc.tensor.matmul(out=mm_y[:], lhsT=wlin_t[:],
                                         rhs=hb2[:], start=True, stop=True)
                        nc.vector.tensor_scalar_add(
                            out=y_t[:, sl], in0=mm_y[:], scalar1=blin_t[:])
                    nc.sync.dma_start(out=ysT_d[t], in_=y_t[:])
    nc.compile()
    return nc


def _warm_devices():
    try:
        import jax
        from jax.sharding import Mesh, PartitionSpec, NamedSharding
        devs = jax.devices()[:NCORES]
        mesh = Mesh(np.asarray(devs), ("core",))
        sh = NamedSharding(mesh, PartitionSpec("core"))
        jax.device_put(np.zeros((NCORES, 4), np.float32), sh).block_until_ready()
    except Exception as e:
        print(f"[kernel] device warm-up failed: {e}", file=sys.stderr)


def kernel(**inputs):
    import threading
    import jax
    jax.devices()          # backend init on the main thread (thread-safety)
    warm = threading.Thread(target=_warm_devices, daemon=True)
    warm.start()
    x = np.asarray(inputs["x"], np.float32)
    eis = np.asarray(inputs["eis"])
    ews = np.asarray(inputs["ews"], np.float32)
    W1 = np.asarray(inputs["W1"], np.float32)
    b1 = np.asarray(inputs["b1"], np.float32)
    W2 = np.asarray(inputs["W2"], np.float32)
    b2 = np.asarray(inputs["b2"], np.float32)
    Wih = np.asarray(inputs["Wih"], np.float32)
    Whh = np.asarray(inputs["Whh"], np.float32)
    bih = np.asarray(inputs["bih"], np.float32)
    bhh = np.asarray(inputs["bhh"], np.float32)
    Wlin = np.asarray(inputs["Wlin"], np.float32)
    blin = np.asarray(inputs["blin"], np.float32)

    _t0 = time.time()
    hp = build_host(x, eis, ews, W1)
    _t0 = _tick("host-prep", _t0)

    nc = build_program(hp)
    _t0 = _tick("build", _t0)

    b1b = np.broadcast_to(b1, (P, H)).copy()
    b2c = b2.reshape(H, 1).copy()
    brc = (bih[:H] + bhh[:H]).reshape(H, 1).copy()
    bzc = (bih[H:2 * H] + bhh[H:2 * H]).reshape(H, 1).copy()
    binc = bih[2 * H:].reshape(H, 1).copy()
    bhnc = bhh[2 * H:].reshape(H, 1).copy()
    blinc = blin.reshape(Z, 1).copy()
    wihT = np.ascontiguousarray(Wih.T).astype(BF)
    whhT = np.ascontiguousarray(Whh.T).astype(BF)
    wlinT = np.ascontiguousarray(Wlin.T).astype(BF)

    in_maps = []
    for c in range(NCORES):
        st = hp["streams"][c]
        tb = hp["tables"][c]
        in_maps.append({
            "xw1bf": hp["xw1bf"],
            "gidx16": wrap16(st["gidx"]),
            "gw128": np.ascontiguousarray(
                st["gw"].reshape(-1, P).T).astype(BF),
            "sidx16": wrap16(st["sidx"]),
            "dinv_blk": tb["dinv_blk"],
            "b1b": b1b, "b2c": b2c, "W2": W2,
            "wihT": wihT, "whhT": whhT, "wlinT": wlinT,
            "br": brc, "bz": bzc, "bin": binc, "bhn": bhnc, "blin": blinc,
        })
    _t0 = _tick("inmaps", _t0)
    warm.join()
    _t0 = _tick("warm-join", _t0)

    res = run_bass_kernel_spmd(nc, in_maps, core_ids=list(range(NCORES)))
    _t0 = _tick("run", _t0)

    out = np.empty((T, N, Z), np.float32)
    for c in range(NCORES):
        lo, hi = c * NSH, min((c + 1) * NSH, N)
        if lo >= N:
            continue
        ys = np.asarray(res.results[c]["ysT"], dtype=np.float32)  # [T, Z, NSH]
        out[:, lo:hi, :] = ys.transpose(0, 2, 1)[:, :hi - lo, :]
    _t0 = _tick("assemble", _t0)
    return out


# revision 12
# speedup vs baseline: 9.8119x; 8.1497x over previous
"""EulerGCN on 8 trn2 NeuronCores — single SPMD launch.

Core t owns snapshot t for the GCN encode: 2 GCN props via ELL gathers +
DVE tree reduce + scatter-add into natural-order DRAM accumulators
(self-loops folded in as ordinary edge tokens; both props share one token
stream since the adjacency is identical). finish1 is a pure streaming
DVE pass; finish2 applies W2 per 128-node block via PE transpose+matmul
and emits tanh(emb)^T feature-major. An in-NEFF AllToAll reshards
feature-major slabs to node-parallel, then a transposed GRU + linear
head run in the same NEFF. Host does integer layout (edge grouping,
degree sort, token grids), GCN normalization, x@W1, and the final
output transpose.
"""

import sys
import time
import numpy as np
import ml_dtypes
import concourse.bass as bass
import concourse.bacc as bacc
import concourse.mybir as mybir
import concourse.tile as tile
from concourse.bass_utils import run_bass_kernel_spmd
from concourse.masks import make_identity

P = 128
NCORES = 8
N = 100000
NPAD = 100352           # 784 blocks of 128
QN = NPAD // 4          # 25088
QBLK = QN // P          # 196
NBLK = NPAD // P        # 784
T = 8
XD = 128
H = 64
Z = 32
NSH = NPAD // NCORES    # 12544
GCH = 448               # GRU chunk cols (28 * 448 = NSH)
NGCH = NSH // GCH
F32 = mybir.dt.float32
BF16 = mybir.dt.bfloat16
I16 = mybir.dt.int16
BF = ml_dtypes.bfloat16

PERF = {}


def _tick(label, t0):
    dt = time.time() - t0
    PERF[label] = PERF.get(label, 0.0) + dt
    print(f"[kernel] {label}: {dt:.2f}s", file=sys.stderr, flush=True)
    return time.time()


def wrap16(a):
    return np.ascontiguousarray(a.reshape(-1, 16).T)


def _prep_core(args):
    eis_c0, eis_c1, ews_c = args
    src = eis_c0.astype(np.int64)
    dst = eis_c1.astype(np.int64)
    w = ews_c.astype(np.float32)
    deg = np.bincount(dst, weights=w, minlength=N).astype(np.float32) + 1.0
    dinv = 1.0 / np.sqrt(deg)                                # [N]
    loops = np.arange(N, dtype=np.int64)
    src = np.concatenate([src, loops])
    dst = np.concatenate([dst, loops])
    wd = np.concatenate([w, np.ones(N, np.float32)]) * dinv[dst]
    sec = (dst // QN) * 4 + (src // QN)
    eo = np.argsort(sec, kind="stable")
    sec_counts = np.bincount(sec, minlength=16)
    src, dst, wd = src[eo], dst[eo], wd[eo]
    bounds = np.concatenate([[0], np.cumsum(sec_counts)])
    secs = []
    for s in range(16):
        r, q = divmod(s, 4)
        sl = slice(bounds[s], bounds[s + 1])
        dl = dst[sl] - r * QN
        s_l = src[sl] - q * QN
        w_l = wd[sl]
        cnt = np.bincount(dl, minlength=QN)
        order = np.argsort(-cnt, kind="stable")              # full QN perm
        rank_of = np.empty(QN, np.int64)
        rank_of[order] = np.arange(QN)
        er = rank_of[dl]
        ei2 = np.argsort(er, kind="stable")
        er_s = er[ei2]
        slot = np.arange(er_s.size) - np.searchsorted(er_s, er_s)
        Ls = cnt[order].reshape(QBLK, P).max(axis=1).astype(np.int64)
        secs.append(dict(er=er_s.astype(np.int32), slot=slot.astype(np.int16),
                         src=s_l[ei2].astype(np.int16), w=w_l[ei2],
                         order=order.astype(np.int32), Ls=Ls))
    return dict(dinv=dinv, secs=secs)


def build_host(x, eis, ews, W1):
    """Per-core tables and shared-shape token grids."""
    xw1 = x.astype(np.float32) @ W1.astype(np.float32)
    percore = [_prep_core((eis[c, 0], eis[c, 1], ews[c]))
               for c in range(T)]

    # common per-section block L (max over cores)
    commonL, nbs = [], []
    for s in range(16):
        Lc = np.zeros(QBLK, np.int64)
        for pc in percore:
            Lc = np.maximum(Lc, pc["secs"][s]["Ls"])
        nz = np.nonzero(Lc)[0]
        nb = int(nz[-1]) + 1 if nz.size else 1
        commonL.append(Lc[:nb])
        nbs.append(nb)
    sec_tok = [int(L.sum()) * P for L in commonL]
    sec_scat = [nb * P for nb in nbs]
    tok_total = sum(sec_tok)
    scat_total = sum(sec_scat)

    offs = [np.concatenate([[0], np.cumsum(Lc)]) * P for Lc in commonL]

    def _streams_core(c):
        g_all = np.zeros(tok_total, np.int16)
        w_all = np.zeros(tok_total, np.float32)
        s_all = np.empty(scat_total, np.int16)
        go = so = 0
        for s in range(16):
            ssec = percore[c]["secs"][s]
            off = offs[s]
            er, slot = ssec["er"], ssec["slot"]
            pos = off[er >> 7] + slot * P + (er & 127)
            g_all[go + pos] = ssec["src"].astype(np.int16)
            w_all[go + pos] = ssec["w"]
            s_all[so:so + sec_scat[s]] = ssec["order"][:sec_scat[s]].astype(np.int16)
            go += sec_tok[s]
            so += sec_scat[s]
        return dict(gidx=g_all, gw=w_all, sidx=s_all)

    streams = [_streams_core(c) for c in range(T)]

    xw1p = np.zeros((NPAD, H), np.float32)
    xw1p[:N] = xw1
    xw1bf = xw1p.astype(BF)
    tables = []
    for c in range(T):
        dpad = np.zeros(NPAD, np.float32)
        dpad[:N] = percore[c]["dinv"]
        tables.append(dict(dinv_blk=dpad.reshape(NBLK, P).T.copy()))
    return dict(commonL=commonL, nbs=nbs, sec_tok=sec_tok, sec_scat=sec_scat,
                tok_total=tok_total, scat_total=scat_total,
                streams=streams, tables=tables, xw1bf=xw1bf)


def build_program(hp):
    commonL = hp["commonL"]
    sec_tok = hp["sec_tok"]
    sec_scat = hp["sec_scat"]
    tok_total = hp["tok_total"]
    scat_total = hp["scat_total"]
    max_tok = max(sec_tok)
    max_scat = max(sec_scat)

    nc = bacc.Bacc(trn_type="TRN2", num_devices=NCORES, num_swdge_queues=4)
    t1bf_d = nc.dram_tensor("xw1bf", [NPAD, H], BF16, kind="ExternalInput")
    gidx_d = nc.dram_tensor("gidx16", [16, tok_total // 16], I16, kind="ExternalInput")
    gw_d = nc.dram_tensor("gw128", [P, tok_total // P], BF16, kind="ExternalInput")
    sidx_d = nc.dram_tensor("sidx16", [16, scat_total // 16], I16, kind="ExternalInput")
    dinv_d = nc.dram_tensor("dinv_blk", [P, NBLK], F32, kind="ExternalInput")
    b1b_d = nc.dram_tensor("b1b", [P, H], F32, kind="ExternalInput")
    b2c_d = nc.dram_tensor("b2c", [H, 1], F32, kind="ExternalInput")
    W2_d = nc.dram_tensor("W2", [H, H], F32, kind="ExternalInput")
    wihT_d = nc.dram_tensor("wihT", [H, 3 * H], BF16, kind="ExternalInput")
    whhT_d = nc.dram_tensor("whhT", [H, 3 * H], BF16, kind="ExternalInput")
    wlinT_d = nc.dram_tensor("wlinT", [H, Z], BF16, kind="ExternalInput")
    br_d = nc.dram_tensor("br", [H, 1], F32, kind="ExternalInput")
    bz_d = nc.dram_tensor("bz", [H, 1], F32, kind="ExternalInput")
    bin_d = nc.dram_tensor("bin", [H, 1], F32, kind="ExternalInput")
    bhn_d = nc.dram_tensor("bhn", [H, 1], F32, kind="ExternalInput")
    blin_d = nc.dram_tensor("blin", [Z, 1], F32, kind="ExternalInput")
    ysT_d = nc.dram_tensor("ysT", [T, Z, NSH], BF16, kind="ExternalOutput")

    table1 = nc.dram_tensor("table1", [NPAD, H], F32)
    table2 = nc.dram_tensor("table2", [NPAD, H], F32)
    acc = [nc.dram_tensor(f"acc{pr}", [NPAD, H], F32) for pr in range(2)]

    with tile.TileContext(nc) as tc:
        with tc.tile_pool(name="const", bufs=1) as cpool, \
             tc.tile_pool(name="dram", bufs=1, space="DRAM") as dpool:
            ident = cpool.tile([P, P], F32)
            make_identity(nc, ident[:])
            dinv_t = cpool.tile([P, NBLK], F32)
            b1_t = cpool.tile([P, H], F32)
            b2c_t = cpool.tile([H, 1], F32)
            W2_t = cpool.tile([H, H], F32)
            wih_t = cpool.tile([H, 3 * H], BF16)
            whh_t = cpool.tile([H, 3 * H], BF16)
            wlin_t = cpool.tile([H, Z], BF16)
            br_t = cpool.tile([H, 1], F32)
            bz_t = cpool.tile([H, 1], F32)
            bin_t = cpool.tile([H, 1], F32)
            bhn_t = cpool.tile([H, 1], F32)
            blin_t = cpool.tile([Z, 1], F32)
            for tt, dd in ((dinv_t, dinv_d), (b1_t, b1b_d), (b2c_t, b2c_d),
                           (W2_t, W2_d), (wih_t, wihT_d), (whh_t, whhT_d),
                           (wlin_t, wlinT_d), (br_t, br_d), (bz_t, bz_d),
                           (bin_t, bin_d), (bhn_t, bhn_d), (blin_t, blin_d)):
                nc.sync.dma_start(out=tt[:], in_=dd[:])

            cc_in = dpool.tile([NCORES * H, NSH], BF16)
            cc_out = dpool.tile([NCORES * H, NSH], BF16)

            # zero accumulators (32 x 0.8MB DMAs)
            zt = cpool.tile([P, 1568], F32)
            nc.gpsimd.memset(zt[:], 0.0)
            for pr in range(2):
                for a0 in range(0, NPAD, 3136):
                    nc.sync.dma_start(out=acc[pr][a0:a0 + 3136, :],
                                      in_=zt[:])

            # expand table1 bf16 -> f32 (8 chunks of 98 blocks)
            with tc.tile_pool(name="exp", bufs=2) as epool:
                for k in range(0, NBLK, 98):
                    src = t1bf_d[k * P:(k + 98) * P, :].rearrange(
                        "(j p) h -> p j h", p=P)
                    tb = epool.tile([P, 98, H], BF16, tag="tbf")
                    nc.sync.dma_start(out=tb[:], in_=src)
                    tf = epool.tile([P, 98, H], F32, tag="tf32")
                    nc.vector.tensor_copy(out=tf[:], in_=tb[:])
                    nc.vector.tensor_tensor(
                        out=tf[:], in0=tf[:],
                        in1=dinv_t[:, k:k + 98].unsqueeze(-1)
                            .broadcast_to([P, 98, H]),
                        op=mybir.AluOpType.mult)
                    nc.sync.dma_start(
                        out=table1[k * P:(k + 98) * P, :].rearrange(
                            "(j p) h -> p j h", p=P),
                        in_=tf[:])

            # ---- the two props ----
            with tc.tile_pool(name="sec", bufs=2) as spool, \
                 tc.tile_pool(name="gath", bufs=3) as gpool:
                qcount = 0
                for pr in range(2):
                    table = table1 if pr == 0 else table2
                    go = so = 0
                    for s in range(16):
                        r, q = divmod(s, 4)
                        Lc = commonL[s]
                        stok, ssc = sec_tok[s], sec_scat[s]
                        if stok == 0:
                            go += stok
                            so += ssc
                            continue
                        gi_b = spool.tile([P, max_tok // 16], I16, tag="gi")
                        si_b = spool.tile([P, max_scat // 16], I16, tag="si")
                        for k in range(8):
                            nc.sync.dma_start(
                                out=gi_b[16 * k:16 * k + 16, :stok // 16],
                                in_=gidx_d[:, go // 16:(go + stok) // 16])
                            nc.sync.dma_start(
                                out=si_b[16 * k:16 * k + 16, :ssc // 16],
                                in_=sidx_d[:, so // 16:(so + ssc) // 16])
                        wbf = spool.tile([P, max_tok // P], BF16, tag="wbf")
                        nc.sync.dma_start(out=wbf[:, :stok // P],
                                          in_=gw_d[:, go // P:(go + stok) // P])
                        w_b = spool.tile([P, max_tok // P], F32, tag="wf")
                        nc.vector.tensor_copy(out=w_b[:, :stok // P],
                                              in_=wbf[:, :stok // P])

                        tbl = table[q * QN:(q + 1) * QN, :]
                        accr = acc[pr][r * QN:(r + 1) * QN, :]
                        lgo = lso = 0   # local token / scatter offsets
                        b = 0
                        while b < len(Lc):
                            L = int(Lc[b])
                            b2 = b
                            while b2 < len(Lc) and int(Lc[b2]) == L:
                                b2 += 1
                            if L == 0:
                                b = b2
                                continue
                            assert L <= 64, L
                            gpc = max(1, 64 // L)
                            bb = b
                            while bb < b2:
                                nbb = min(gpc, b2 - bb)
                                ncols = nbb * L
                                tok = ncols * P
                                stk = nbb * P
                                pk = gpool.tile([P, 64, H], F32, tag="pk")
                                if L == 1:
                                    gt = gpool.tile([P, 64, H], F32, tag="g")
                                    nc.gpsimd.dma_gather(
                                        out_ap=gt[:, :ncols, :], in_ap=tbl,
                                        idxs_ap=gi_b[:, lgo // 16:(lgo + tok) // 16],
                                        num_idxs=tok, num_idxs_reg=tok,
                                        elem_size=H, single_packet=False,
                                        queue_num=qcount % 4)
                                    nc.vector.tensor_tensor(
                                        out=pk[:, :ncols, :], in0=gt[:, :ncols, :],
                                        in1=w_b[:, lgo // P:lgo // P + ncols]
                                            .unsqueeze(-1)
                                            .broadcast_to([P, ncols, H]),
                                        op=mybir.AluOpType.mult)
                                else:
                                    gt = gpool.tile([P, 64, H], F32, tag="g")
                                    nc.gpsimd.dma_gather(
                                        out_ap=gt[:, :ncols, :], in_ap=tbl,
                                        idxs_ap=gi_b[:, lgo // 16:(lgo + tok) // 16],
                                        num_idxs=tok, num_idxs_reg=tok,
                                        elem_size=H, single_packet=False,
                                        queue_num=qcount % 4)
                                    nc.vector.tensor_tensor(
                                        out=gt[:, :ncols, :], in0=gt[:, :ncols, :],
                                        in1=w_b[:, lgo // P:lgo // P + ncols]
                                            .unsqueeze(-1)
                                            .broadcast_to([P, ncols, H]),
                                        op=mybir.AluOpType.mult)
                                    gv = gt[:, :ncols, :].rearrange(
                                        "p (g l) h -> p g l h", l=L)
                                    width = L
                                    while width > 2:
                                        half = width // 2
                                        nc.vector.tensor_tensor(
                                            out=gv[:, :, :half, :],
                                            in0=gv[:, :, :half, :],
                                            in1=gv[:, :, width - half:width, :],
                                            op=mybir.AluOpType.add)
                                        width -= half
                                    if width == 2:
                                        nc.vector.tensor_tensor(
                                            out=pk[:, :nbb, :],
                                            in0=gv[:, :, 0, :], in1=gv[:, :, 1, :],
                                            op=mybir.AluOpType.add)
                                    else:
                                        nc.vector.tensor_copy(
                                            out=pk[:, :nbb, :], in_=gv[:, :, 0, :])
                                nc.gpsimd.dma_scatter_add(
                                    accr, pk[:, :nbb, :],
                                    si_b[:, lso // 16:(lso + stk) // 16],
                                    stk, stk, H, queue_num=qcount % 4)
                                qcount += 1
                                lgo += tok
                                lso += stk
                                bb += nbb
                            b = b2
                        go += stok
                        so += ssc

                    # ---- finish pass ----
                    if pr == 0:
                        with tc.tile_pool(name="fin", bufs=2) as fpool:
                            for k in range(0, NBLK, 49):
                                av = fpool.tile([P, 49, H], F32, tag="av")
                                nc.sync.dma_start(
                                    out=av[:],
                                    in_=acc[0][k * P:(k + 49) * P, :].rearrange(
                                        "(j p) h -> p j h", p=P))
                                nc.vector.tensor_tensor(
                                    out=av[:], in0=av[:],
                                    in1=b1_t[:].unsqueeze(1)
                                        .broadcast_to([P, 49, H]),
                                    op=mybir.AluOpType.add)
                                nc.vector.tensor_scalar_max(
                                    out=av[:], in0=av[:], scalar1=0.0)
                                nc.vector.tensor_tensor(
                                    out=av[:], in0=av[:],
                                    in1=dinv_t[:, k:k + 49].unsqueeze(-1)
                                        .broadcast_to([P, 49, H]),
                                    op=mybir.AluOpType.mult)
                                nc.sync.dma_start(
                                    out=table2[k * P:(k + 49) * P, :].rearrange(
                                        "(j p) h -> p j h", p=P),
                                    in_=av[:])

            # ---- finish2: W2, bias, tanh, transpose to feature-major ----
            with tc.tile_pool(name="f2", bufs=3) as f2pool, \
                 tc.tile_pool(name="f2p", bufs=4, space="PSUM") as f2ps:
                for j in range(NCORES):          # peer slab
                    for c0 in range(0, 98, 8):
                        nb2 = min(8, 98 - c0)
                        k0 = j * 98 + c0
                        av2 = f2pool.tile([P, 8, H], F32, tag="av2")
                        nc.sync.dma_start(
                            out=av2[:, :nb2, :],
                            in_=acc[1][k0 * P:(k0 + nb2) * P, :].rearrange(
                                "(j p) h -> p j h", p=P))
                        for g0 in range(0, nb2, 4):
                            ng = min(4, nb2 - g0)
                            eg = f2pool.tile([H, 4 * P], BF16, tag="eg")
                            for bi in range(ng):
                                pt = f2ps.tile([H, P], F32, tag="pt")
                                nc.tensor.transpose(
                                    out=pt[:], in_=av2[:, g0 + bi, :],
                                    identity=ident[:])
                                abT = f2pool.tile([H, P], F32, tag="abT")
                                nc.vector.tensor_copy(out=abT[:], in_=pt[:])
                                mm = f2ps.tile([H, P], F32, tag="mm")
                                nc.tensor.matmul(out=mm[:], lhsT=W2_t[:],
                                                 rhs=abT[:], start=True, stop=True)
                                nc.scalar.activation(
                                    out=eg[:, bi * P:(bi + 1) * P], in_=mm[:],
                                    func=mybir.ActivationFunctionType.Tanh,
                                    bias=b2c_t[:])
                            cz = (c0 + g0) * P
                            nc.sync.dma_start(
                                out=cc_in[j * H:(j + 1) * H, cz:cz + ng * P],
                                in_=eg[:, :ng * P])

            # ---- AllToAll reshard ----
            nc.gpsimd.collective_compute(
                "AllToAll", mybir.AluOpType.bypass,
                replica_groups=[list(range(NCORES))],
                ins=[cc_in[:]], outs=[cc_out[:]])

            # ---- GRU + head (transposed layout) ----
            with tc.tile_pool(name="gs", bufs=1) as gspool, \
                 tc.tile_pool(name="gx", bufs=2) as gxpool, \
                 tc.tile_pool(name="gw", bufs=2) as gwpool, \
                 tc.tile_pool(name="gp", bufs=2, space="PSUM") as gppool, \
                 tc.tile_pool(name="gp1", bufs=1, space="PSUM") as gppool1:
                h32 = gspool.tile([H, NSH], F32)
                nc.gpsimd.memset(h32[:], 0.0)
                for t in range(T):
                    xsT = gxpool.tile([H, NSH], BF16, tag="xs")
                    nc.sync.dma_start(out=xsT[:],
                                      in_=cc_out[t * H:(t + 1) * H, :])
                    y_t = gxpool.tile([Z, NSH], BF16, tag="y")
                    for i in range(NGCH):
                        sl = slice(i * GCH, (i + 1) * GCH)
                        hb = gwpool.tile([H, GCH], BF16, tag="hb")
                        nc.vector.tensor_copy(out=hb[:], in_=h32[:, sl])
                        mm_r = gppool.tile([H, GCH], F32, tag="mr")
                        nc.tensor.matmul(out=mm_r[:], lhsT=wih_t[:, :H],
                                         rhs=xsT[:, sl], start=True, stop=False)
                        nc.tensor.matmul(out=mm_r[:], lhsT=whh_t[:, :H],
                                         rhs=hb[:], start=False, stop=True)
                        mm_z = gppool.tile([H, GCH], F32, tag="mz")
                        nc.tensor.matmul(out=mm_z[:], lhsT=wih_t[:, H:P],
                                         rhs=xsT[:, sl], start=True, stop=False)
                        nc.tensor.matmul(out=mm_z[:], lhsT=whh_t[:, H:P],
                                         rhs=hb[:], start=False, stop=True)
                        r_sb = gwpool.tile([H, GCH], F32, tag="r")
                        nc.scalar.activation(
                            out=r_sb[:], in_=mm_r[:],
                            func=mybir.ActivationFunctionType.Sigmoid,
                            bias=br_t[:])
                        z_sb = gwpool.tile([H, GCH], F32, tag="z")
                        nc.scalar.activation(
                            out=z_sb[:], in_=mm_z[:],
                            func=mybir.ActivationFunctionType.Sigmoid,
                            bias=bz_t[:])
                        mm_hn = gppool1.tile([H, GCH], F32, tag="mhn")
                        nc.tensor.matmul(out=mm_hn[:], lhsT=whh_t[:, P:],
                                         rhs=hb[:], start=True, stop=True)
                        rn = gwpool.tile([H, GCH], F32, tag="rn")
                        nc.vector.tensor_scalar_add(
                            out=rn[:], in0=mm_hn[:], scalar1=bhn_t[:])
                        nc.vector.tensor_tensor(
                            out=rn[:], in0=rn[:], in1=r_sb[:],
                            op=mybir.AluOpType.mult)
                        mm_in = gppool1.tile([H, GCH], F32, tag="min")
                        nc.tensor.matmul(out=mm_in[:], lhsT=wih_t[:, P:],
                                         rhs=xsT[:, sl], start=True, stop=True)
                        npre = gwpool.tile([H, GCH], F32, tag="npre")
                        nc.vector.tensor_tensor(
                            out=npre[:], in0=mm_in[:], in1=rn[:],
                            op=mybir.AluOpType.add)
                        n_sb = gwpool.tile([H, GCH], F32, tag="nsb")
                        nc.scalar.activation(
                            out=n_sb[:], in_=npre[:],
                            func=mybir.ActivationFunctionType.Tanh,
                            bias=bin_t[:])
                        d = gwpool.tile([H, GCH], F32, tag="d")
                        nc.vector.tensor_tensor(
                            out=d[:], in0=h32[:, sl], in1=n_sb[:],
                            op=mybir.AluOpType.subtract)
                        nc.vector.tensor_tensor(
                            out=d[:], in0=d[:], in1=z_sb[:],
                            op=mybir.AluOpType.mult)
                        nc.vector.tensor_tensor(
                            out=h32[:, sl], in0=n_sb[:], in1=d[:],
                            op=mybir.AluOpType.add)
                        hb2 = gwpool.tile([H, GCH], BF16, tag="hb2")
                        nc.vector.tensor_copy(out=hb2[:], in_=h32[:, sl])
                        mm_y = gppool.tile([Z, GCH], F32, tag="my")
                        nc.tensor.matmul(out=mm_y[:], lhsT=wlin_t[:],
                                         rhs=hb2[:], start=True, stop=True)
                        nc.vector.tensor_scalar_add(
                            out=y_t[:, sl], in0=mm_y[:], scalar1=blin_t[:])
                    nc.sync.dma_start(out=ysT_d[t], in_=y_t[:])
    nc.compile()
    return nc


def _warm_devices():
    try:
        import jax
        from jax.sharding import Mesh, PartitionSpec, NamedSharding
        devs = jax.devices()[:NCORES]
        mesh = Mesh(np.asarray(devs), ("core",))
        sh = NamedSharding(mesh, PartitionSpec("core"))
        jax.device_put(np.zeros((NCORES, 4), np.float32), sh).block_until_ready()
    except Exception as e:
        print(f"[kernel] device warm-up failed: {e}", file=sys.stderr)


def _start_warm():
    try:
        import threading
        import jax
        jax.devices()      # backend init on the importing thread
        th = threading.Thread(target=_warm_devices, daemon=True)
        th.start()
        return th
    except Exception as e:
        print(f"[kernel] warm start failed: {e}", file=sys.stderr)
        return None


_WARM = _start_warm()


def kernel(**inputs):
    warm = _WARM if _WARM is not None else _start_warm()
    x = np.asarray(inputs["x"], np.float32)
    eis = np.asarray(inputs["eis"])
    ews = np.asarray(inputs["ews"], np.float32)
    W1 = np.asarray(inputs["W1"], np.float32)
    b1 = np.asarray(inputs["b1"], np.float32)
    W2 = np.asarray(inputs["W2"], np.float32)
    b2 = np.asarray(inputs["b2"], np.float32)
    Wih = np.asarray(inputs["Wih"], np.float32)
    Whh = np.asarray(inputs["Whh"], np.float32)
    bih = np.asarray(inputs["bih"], np.float32)
    bhh = np.asarray(inputs["bhh"], np.float32)
    Wlin = np.asarray(inputs["Wlin"], np.float32)
    blin = np.asarray(inputs["blin"], np.float32)

    _t0 = time.time()
    hp = build_host(x, eis, ews, W1)
    _t0 = _tick("host-prep", _t0)

    nc = build_program(hp)
    _t0 = _tick("build", _t0)

    b1b = np.broadcast_to(b1, (P, H)).copy()
    b2c = b2.reshape(H, 1).copy()
    brc = (bih[:H] + bhh[:H]).reshape(H, 1).copy()
    bzc = (bih[H:2 * H] + bhh[H:2 * H]).reshape(H, 1).copy()
    binc = bih[2 * H:].reshape(H, 1).copy()
    bhnc = bhh[2 * H:].reshape(H, 1).copy()
    blinc = blin.reshape(Z, 1).copy()
    wihT = np.ascontiguousarray(Wih.T).astype(BF)
    whhT = np.ascontiguousarray(Whh.T).astype(BF)
    wlinT = np.ascontiguousarray(Wlin.T).astype(BF)

    in_maps = []
    for c in range(NCORES):
        st = hp["streams"][c]
        tb = hp["tables"][c]
        in_maps.append({
            "xw1bf": hp["xw1bf"],
            "gidx16": wrap16(st["gidx"]),
            "gw128": np.ascontiguousarray(
                st["gw"].reshape(-1, P).T).astype(BF),
            "sidx16": wrap16(st["sidx"]),
            "dinv_blk": tb["dinv_blk"],
            "b1b": b1b, "b2c": b2c, "W2": W2,
            "wihT": wihT, "whhT": whhT, "wlinT": wlinT,
            "br": brc, "bz": bzc, "bin": binc, "bhn": bhnc, "blin": blinc,
        })
    _t0 = _tick("inmaps", _t0)
    if warm is not None:
        warm.join()
    _t0 = _tick("warm-join", _t0)

    res = run_bass_kernel_spmd(nc, in_maps, core_ids=list(range(NCORES)))
    _t0 = _tick("run", _t0)

    out = np.empty((T, N, Z), np.float32)
    for c in range(NCORES):
        lo, hi = c * NSH, min((c + 1) * NSH, N)
        if lo >= N:
            continue
        ys = np.asarray(res.results[c]["ysT"], dtype=np.float32)  # [T, Z, NSH]
        out[:, lo:hi, :] = ys.transpose(0, 2, 1)[:, :hi - lo, :]
    _t0 = _tick("assemble", _t0)
    return out
